# revision 2
# baseline (speedup 1.0000x reference)
"""CostDifference kernel v2c for Trainium2 (Bass/Tile), 8-core SPMD.

out[n, d, c, h, w] = left[n,c,h,w] - right[n,c,h+s,w] for h+s < H else 0,
s = 128 - d. Channel-parallel: core k handles channels {2k, 2k+1}.

Design (the cost model prices DMA as one exclusive serial device at
~360GB/s and compute by free-axis size only — partitions are free):
- ra fine-shift table (right shifted by r=1..32): bands r=9..32 built by
  the TENSOR engine (matmul vs a diagonal-constant free-slice: a shift
  with no partition-alignment rules) into PSUM, copied to SBUF by
  Activation; band r=1..8 loaded from HBM. Saves ~8MB of table DMA.
- PE/ACT lane: q=1 disparities d in [64, 96) (whose quadrant span rules
  would force 2 DVE ops each) are matmul pairs (psum = I*left - I_s*right)
  drained psum->sbuf by ACT (a few by DVE).
- PE/DMA lane: small disparities d in [1, 24) are matmul pairs stored
  DIRECTLY from PSUM to HBM as fp32 (separate out32 tensor, merged on
  host). Removes their subs from DVE/Pool entirely.
- 4 ring tiles with an explicit ascending-use map: tail-triangle garbage
  cells are never overwritten by earlier groups on the same tile, so a
  single startup zeroing per tile keeps them zero forever (they land on
  the pre-zeroed output region).
All math fp16 (gate is 2e-2 l2; fp16 gives ~3e-4), host upcasts.
"""

import sys

sys.path.insert(0, "/opt/trn_rl_repo")

import numpy as np

import concourse.bacc as bacc
from concourse.bass import AP
import concourse.mybir as mybir
from concourse import tile
from concourse.bass_utils import run_bass_kernel_spmd

N, C, H, W = 1, 16, 128, 256
D = 128                      # disparities; d has shift s = 128 - d
N_CORES = 8
C_LOC = C // N_CORES         # channels per core (2)
F = C_LOC * W                # free elems per disparity row (512)
G = 8                        # disparities per batched store
NQ = 32                      # fine-shift chunks, r in [1, 32]
LPAD = 96                    # zero rows atop left (coarse down-shifts)
RPAD = 32                    # zero rows below right (fine up-shifts)

CFG = {
    "pool_share": 0.30,       # fraction of DVE/Pool subs on Pool
    "q1_dve": {67, 71, 75, 79, 83, 87, 91, 95},  # q1 drained by DVE not ACT
    "k3max": 1,              # d in [1, k3max) go via the PE/DMA psum lane
    "mz": ("gpsimd", "vector", "gpsimd", "vector"),
    "act_extra": (),
    "sched": [(24, 0), (16, 8), (None, 12), (None, 4),
              (None, 1), (None, 9), (None, 13), (None, 5),
              (8, 2), (None, 10), (None, 14), (None, 6),
              (None, 11), (None, 15), (None, 7), (None, 3)],
    "tmap": {0: 0, 8: 1, 12: 2, 4: 3, 1: 0, 9: 1, 13: 2, 5: 3,
             2: 0, 10: 1, 14: 2, 6: 3, 3: 0, 11: 1, 15: 2, 7: 3},
}

_cached = {}


def _build_program():
    f16 = mybir.dt.float16
    f32 = mybir.dt.float32
    CPY = mybir.ActivationFunctionType.Copy
    nc = bacc.Bacc("TRN2", target_bir_lowering=False, debug=False,
                   enable_asserts=False, num_devices=N_CORES)
    lh = nc.dram_tensor("left", [LPAD + H, F], f16, kind="ExternalInput")
    rh = nc.dram_tensor("right", [H + RPAD, F], f16, kind="ExternalInput")
    ewh = nc.dram_tensor("ew", [128, 256], f16, kind="ExternalInput")
    ewnh = nc.dram_tensor("ewn", [128, 256], f16, kind="ExternalInput")
    oh = nc.dram_tensor("out", [D, H, F], f16, kind="ExternalOutput")
    K3 = CFG["k3max"]
    o32 = nc.dram_tensor("out32", [K3, 32, F], f32, kind="ExternalOutput")

    act_ds = set(range(64, 96)) | set(CFG.get("act_extra", ()))
    k3_ds = set(range(1, K3))     # PE/DMA lane

    with tile.TileContext(nc) as tc:
        with tc.tile_pool(name="sbuf", bufs=1) as pool, \
             tc.tile_pool(name="pp", bufs=1, space="PSUM") as ppool:
            lc = pool.tile([H, 4 * F], f16, tag="lc")
            ra = pool.tile([H, NQ * F], f16, tag="ra")
            rt = pool.tile([H, F], f16, tag="rt")
            ewt = pool.tile([128, 256], f16, tag="ewt")
            ewnt = pool.tile([128, 256], f16, tag="ewnt")
            zq = pool.tile([128, F], f16, tag="zq")
            rings = [pool.tile([H, G * F], f16, name=f"rg{b}", tag=f"rg{b}")
                     for b in range(4)]
            ptabs = [ppool.tile([128, 2 * F], f32, name=f"pt{i}", tag=f"pt{i}")
                     for i in range(2)]
            pq = [ppool.tile([128, F], f32, name=f"pq{i}", tag=f"pq{i}")
                  for i in range(4)]

            # loads (SP ring)
            nc.sync.dma_start(out=ewt[:], in_=AP(ewh, 0, [[256, 128], [1, 256]]))
            nc.sync.dma_start(out=ewnt[:], in_=AP(ewnh, 0, [[256, 128], [1, 256]]))
            nc.sync.dma_start(out=rt[:], in_=AP(rh, 0, [[F, H], [1, F]]))
            # lc[p, j, :] = lh[p + 32j, :]
            nc.sync.dma_start(
                out=lc[:].rearrange("p (j f) -> p j f", j=4),
                in_=AP(lh, 0, [[F, H], [32 * F, 4], [1, F]]))
            # band r=1..8 (chunks 0-7) via DMA; consumed by g3, g15, g7
            nc.sync.dma_start(
                out=ra[0:127, 0:8 * F].rearrange("p (i f) -> p i f", i=8),
                in_=AP(rh, F, [[F, 127], [F, 8], [1, F]]))

            # one-time ring zeroing, ordered by first use
            need_zq = "scalar" in CFG["mz"]
            if need_zq:
                nc.vector.memset(zq[:], 0.0)
            for b in range(4):
                if CFG["mz"][b] == "scalar":
                    continue  # zeroed later in the ACT stream (see sched loop)
                getattr(nc, CFG["mz"][b]).memset(rings[b][:], 0.0)

            pool_acc = 0.0

            def pick_engine():
                nonlocal pool_acc
                pool_acc += CFG["pool_share"]
                if pool_acc >= 1.0:
                    pool_acc -= 1.0
                    return nc.gpsimd
                return nc.vector

            def build_band(i0):
                # entries r = i0+1 .. i0+8 into ra chunks i0..i0+7
                for quad in range(4):
                    pt = ptabs[quad % 2]
                    for k in range(2):
                        i = i0 + 2 * quad + k
                        r = i + 1
                        nc.tensor.matmul(out=pt[0:128 - r, k * F:(k + 1) * F],
                                         lhsT=ewt[:, 96 + r:224],
                                         rhs=rt[:], start=True, stop=True)
                    nc.scalar.activation(
                        out=ra[:, (i0 + 2 * quad) * F:(i0 + 2 * quad + 2) * F],
                        in_=pt[:], func=CPY)

            npq = [0]

            def mm_pair(pb, d, s):
                # psum rows [0,d) = left - right_shift(s)
                nc.tensor.matmul(out=pb[0:d, :], lhsT=ewt[:, 96:96 + d],
                                 rhs=lc[:, 3 * F:4 * F], start=True, stop=False)
                nc.tensor.matmul(out=pb[0:d, :],
                                 lhsT=ewnt[:, 96 + s:96 + s + d],
                                 rhs=rt[:], start=False, stop=True)

            def do_k3(ds):
                # PE/DMA lane: psum -> fp32 store of rows [0, d)
                for d in ds:
                    pb = pq[npq[0] % 4]
                    npq[0] += 1
                    mm_pair(pb, d, D - d)
                    nc.sync.dma_start(out=AP(o32, d * 32 * F, [[F, d], [1, F]]),
                                      in_=pb[0:d, :])

            def do_group(g, tq):
                R = g * G + G
                base = g * G * H * F
                for j in range(G):
                    d = g * G + j
                    if d == 0 or d in k3_ds:
                        continue  # chunk stays zero (k3 data goes via out32)
                    s = D - d
                    if d in act_ds:
                        pb = pq[npq[0] % 4]
                        npq[0] += 1
                        mm_pair(pb, d, s)
                        dst = tq[0:d, j * F:(j + 1) * F]
                        if d in CFG["q1_dve"]:
                            nc.vector.tensor_copy(out=dst, in_=pb[0:d, :])
                        else:
                            nc.scalar.activation(out=dst, in_=pb[0:d, :],
                                                 func=CPY)
                        continue
                    q = (s - 1) // 32
                    r = s - 32 * q          # in [1, 32]
                    b = 32 * q
                    pick_engine().tensor_sub(
                        out=tq[0:d, j * F:(j + 1) * F],
                        in0=lc[b:b + d, (3 - q) * F:(4 - q) * F],
                        in1=ra[b:b + d, (r - 1) * F:r * F])
                # one rect store [0, R) x 8 chunks; tail-triangle cells are
                # pre-zeroed and land on the output's zero region
                nc.sync.dma_start(
                    out=AP(oh, base, [[F, R], [H * F, G], [1, F]]),
                    in_=tq[0:R, :].rearrange("p (j f) -> p j f", j=G))

            # schedule: explicit tile map, ascending groups per tile.
            # bands: 24 -> g12, g4; 16 -> g13, g5; 8 -> g14, g6;
            # DMA band (r=1..8) -> g3, g15, g7; q1 groups 8-11 need no band.
            tmap = CFG["tmap"]
            zeroed = set()
            for pre, g in CFG["sched"]:
                if pre is not None:
                    build_band(pre)
                    for b in range(4):
                        if CFG["mz"][b] == "scalar" and b not in zeroed:
                            zeroed.add(b)
                            nc.scalar.activation(
                                out=rings[b][:].rearrange(
                                    "p (j f) -> p j f", j=G),
                                in_=zq[:].unsqueeze(1)
                                .broadcast_to((128, G, F)), func=CPY)
                            break
                do_group(g, rings[tmap[g]])
    nc.compile()
    return nc


def _make_ew():
    ew = np.zeros((128, 256), np.float16)
    ewn = np.zeros((128, 256), np.float16)
    for p in range(128):
        ew[p, p + 96] = 1.0
        ewn[p, p + 96] = -1.0
    return ew, ewn


def _run(left, right, trace=False):
    """left/right: [N, C, H, W] f32. Returns (full_out, exec_time_ns)."""
    if "nc" not in _cached:
        _cached["nc"] = _build_program()
    nc = _cached["nc"]
    left = np.asarray(left)
    right = np.asarray(right)
    ew, ewn = _make_ew()
    in_maps = []
    for k in range(N_CORES):
        sl = slice(k * C_LOC, (k + 1) * C_LOC)
        lt = left[0, sl].transpose(1, 0, 2).reshape(H, F).astype(np.float16)
        rtm = right[0, sl].transpose(1, 0, 2).reshape(H, F).astype(np.float16)
        lp = np.concatenate([np.zeros((LPAD, F), np.float16), lt], axis=0)
        rp = np.concatenate([rtm, np.zeros((RPAD, F), np.float16)], axis=0)
        in_maps.append({"left": np.ascontiguousarray(lp),
                        "right": np.ascontiguousarray(rp),
                        "ew": ew, "ewn": ewn})
    res = run_bass_kernel_spmd(nc, in_maps, core_ids=list(range(N_CORES)),
                               trace=trace)
    K3 = CFG["k3max"]
    parts = []
    for k in range(N_CORES):
        full16 = (res.results[k]["out"].astype(np.float32)
                  .reshape(D, H, C_LOC, W))
        o32 = res.results[k]["out32"].reshape(K3, 32, C_LOC, W)
        for d in range(1, K3):
            full16[d, 0:d] = o32[d, 0:d]
        parts.append(full16.transpose(0, 2, 1, 3))
    full = np.concatenate(parts, axis=1)
    return np.ascontiguousarray(full[None]), res.exec_time_ns


def kernel(left, right):
    out, _ = _run(left, right, trace=False)
    return out


# revision 3
# speedup vs baseline: 1.0106x; 1.0106x over previous
"""CostDifference kernel v2c for Trainium2 (Bass/Tile), 8-core SPMD.

out[n, d, c, h, w] = left[n,c,h,w] - right[n,c,h+s,w] for h+s < H else 0,
s = 128 - d. Channel-parallel: core k handles channels {2k, 2k+1}.

Design (the cost model prices DMA as one exclusive serial device at
~360GB/s and compute by free-axis size only — partitions are free):
- ra fine-shift table (right shifted by r=1..32): bands r=9..32 built by
  the TENSOR engine (matmul vs a diagonal-constant free-slice: a shift
  with no partition-alignment rules) into PSUM, copied to SBUF by
  Activation; band r=1..8 loaded from HBM. Saves ~8MB of table DMA.
- PE/ACT lane: q=1 disparities d in [64, 96) (whose quadrant span rules
  would force 2 DVE ops each) are matmul pairs (psum = I*left - I_s*right)
  drained psum->sbuf by ACT (a few by DVE).
- (A PSUM->HBM direct-store lane was tried and reverted: the DMA engine
  cannot read PSUM, so k3max=1 disables it.)
- 4 ring tiles with an explicit ascending-use map: tail-triangle garbage
  cells are never overwritten by earlier groups on the same tile, so a
  single startup zeroing per tile keeps them zero forever (they land on
  the pre-zeroed output region).
All math fp16 (gate is 2e-2 l2; fp16 gives ~3e-4), host upcasts.
"""

import sys

sys.path.insert(0, "/opt/trn_rl_repo")

import numpy as np

import concourse.bacc as bacc
from concourse.bass import AP
import concourse.mybir as mybir
from concourse import tile
from concourse.bass_utils import run_bass_kernel_spmd

N, C, H, W = 1, 16, 128, 256
D = 128                      # disparities; d has shift s = 128 - d
N_CORES = 8
C_LOC = C // N_CORES         # channels per core (2)
F = C_LOC * W                # free elems per disparity row (512)
G = 8                        # disparities per batched store
NQ = 32                      # fine-shift chunks, r in [1, 32]
LPAD = 96                    # zero rows atop left (coarse down-shifts)
RPAD = 32                    # zero rows below right (fine up-shifts)

CFG = {
    "pool_share": 0.30,       # fraction of DVE/Pool subs on Pool
    "q1_dve": {67, 71, 75, 79, 83, 87, 91, 95},  # q1 drained by DVE not ACT
    "k3max": 1,              # d in [1, k3max) go via the PE/DMA psum lane
    "mz": ("gpsimd", "vector", "gpsimd", "vector"),
    "act_extra": (),
    "sched": [(24, 0), (16, 8), (None, 12), (None, 4),
              (None, 1), (None, 9), (None, 13), (None, 5),
              (8, 2), (None, 10), (None, 14), (None, 6),
              (None, 11), (None, 15), (None, 7), (None, 3)],
    "tmap": {0: 0, 8: 1, 12: 2, 4: 3, 1: 0, 9: 1, 13: 2, 5: 3,
             2: 0, 10: 1, 14: 2, 6: 3, 3: 0, 11: 1, 15: 2, 7: 3},
}

_cached = {}


def _build_program():
    f16 = mybir.dt.float16
    f32 = mybir.dt.float32
    CPY = mybir.ActivationFunctionType.Copy
    nc = bacc.Bacc("TRN2", target_bir_lowering=False, debug=False,
                   enable_asserts=False, num_devices=N_CORES)
    lh = nc.dram_tensor("left", [LPAD + H, F], f16, kind="ExternalInput")
    rh = nc.dram_tensor("right", [H + RPAD, F], f16, kind="ExternalInput")
    ewh = nc.dram_tensor("ew", [128, 256], f16, kind="ExternalInput")
    ewnh = nc.dram_tensor("ewn", [128, 256], f16, kind="ExternalInput")
    oh = nc.dram_tensor("out", [D, H, F], f16, kind="ExternalOutput")
    K3 = CFG["k3max"]
    o32 = nc.dram_tensor("out32", [K3, 32, F], f32, kind="ExternalOutput")

    act_ds = set(range(64, 96)) | set(CFG.get("act_extra", ()))
    k3_ds = set(range(1, K3))     # PE/DMA lane

    with tile.TileContext(nc) as tc:
        with tc.tile_pool(name="sbuf", bufs=1) as pool, \
             tc.tile_pool(name="pp", bufs=1, space="PSUM") as ppool:
            lc = pool.tile([H, 4 * F], f16, tag="lc")
            ra = pool.tile([H, NQ * F], f16, tag="ra")
            rt = pool.tile([H, F], f16, tag="rt")
            ewt = pool.tile([128, 256], f16, tag="ewt")
            ewnt = pool.tile([128, 256], f16, tag="ewnt")
            zq = pool.tile([128, F], f16, tag="zq")
            rings = [pool.tile([H, G * F], f16, name=f"rg{b}", tag=f"rg{b}")
                     for b in range(4)]
            ptabs = [ppool.tile([128, 2 * F], f32, name=f"pt{i}", tag=f"pt{i}")
                     for i in range(2)]
            pq = [ppool.tile([128, F], f32, name=f"pq{i}", tag=f"pq{i}")
                  for i in range(4)]

            # loads (SP ring)
            nc.sync.dma_start(out=ewt[:], in_=AP(ewh, 0, [[256, 128], [1, 256]]))
            nc.sync.dma_start(out=ewnt[:], in_=AP(ewnh, 0, [[256, 128], [1, 256]]))
            nc.sync.dma_start(out=rt[:], in_=AP(rh, 0, [[F, H], [1, F]]))
            # lc[p, j, :] = lh[p + 32j, :]
            nc.sync.dma_start(
                out=lc[:].rearrange("p (j f) -> p j f", j=4),
                in_=AP(lh, 0, [[F, H], [32 * F, 4], [1, F]]))
            # band r=1..8 (chunks 0-7) via DMA; consumed by g3, g15, g7
            nc.sync.dma_start(
                out=ra[0:127, 0:8 * F].rearrange("p (i f) -> p i f", i=8),
                in_=AP(rh, F, [[F, 127], [F, 8], [1, F]]))

            # one-time ring zeroing, ordered by first use
            need_zq = "scalar" in CFG["mz"]
            if need_zq:
                nc.vector.memset(zq[:], 0.0)
            for b in range(4):
                if CFG["mz"][b] == "scalar":
                    continue  # zeroed later in the ACT stream (see sched loop)
                getattr(nc, CFG["mz"][b]).memset(rings[b][:], 0.0)

            pool_acc = 0.0

            def pick_engine():
                nonlocal pool_acc
                pool_acc += CFG["pool_share"]
                if pool_acc >= 1.0:
                    pool_acc -= 1.0
                    return nc.gpsimd
                return nc.vector

            def build_band(i0):
                # entries r = i0+1 .. i0+8 into ra chunks i0..i0+7
                for quad in range(4):
                    pt = ptabs[quad % 2]
                    for k in range(2):
                        i = i0 + 2 * quad + k
                        r = i + 1
                        nc.tensor.matmul(out=pt[0:128 - r, k * F:(k + 1) * F],
                                         lhsT=ewt[:, 96 + r:224],
                                         rhs=rt[:], start=True, stop=True)
                    nc.scalar.activation(
                        out=ra[:, (i0 + 2 * quad) * F:(i0 + 2 * quad + 2) * F],
                        in_=pt[:], func=CPY)

            npq = [0]

            def mm_pair(pb, d, s):
                # psum rows [0,d) = left - right_shift(s)
                nc.tensor.matmul(out=pb[0:d, :], lhsT=ewt[:, 96:96 + d],
                                 rhs=lc[:, 3 * F:4 * F], start=True, stop=False)
                nc.tensor.matmul(out=pb[0:d, :],
                                 lhsT=ewnt[:, 96 + s:96 + s + d],
                                 rhs=rt[:], start=False, stop=True)

            def do_k3(ds):
                # PE/DMA lane: psum -> fp32 store of rows [0, d)
                for d in ds:
                    pb = pq[npq[0] % 4]
                    npq[0] += 1
                    mm_pair(pb, d, D - d)
                    nc.sync.dma_start(out=AP(o32, d * 32 * F, [[F, d], [1, F]]),
                                      in_=pb[0:d, :])

            def do_group(g, tq):
                R = g * G + G
                base = g * G * H * F
                for j in range(G):
                    d = g * G + j
                    if d == 0 or d in k3_ds:
                        continue  # chunk stays zero (k3 data goes via out32)
                    s = D - d
                    if d in act_ds:
                        pb = pq[npq[0] % 4]
                        npq[0] += 1
                        mm_pair(pb, d, s)
                        dst = tq[0:d, j * F:(j + 1) * F]
                        if d in CFG["q1_dve"]:
                            nc.vector.tensor_copy(out=dst, in_=pb[0:d, :])
                        else:
                            nc.scalar.activation(out=dst, in_=pb[0:d, :],
                                                 func=CPY)
                        continue
                    q = (s - 1) // 32
                    r = s - 32 * q          # in [1, 32]
                    b = 32 * q
                    pick_engine().tensor_sub(
                        out=tq[0:d, j * F:(j + 1) * F],
                        in0=lc[b:b + d, (3 - q) * F:(4 - q) * F],
                        in1=ra[b:b + d, (r - 1) * F:r * F])
                # one rect store [0, R) x 8 chunks; tail-triangle cells are
                # pre-zeroed and land on the output's zero region
                nc.sync.dma_start(
                    out=AP(oh, base, [[F, R], [H * F, G], [1, F]]),
                    in_=tq[0:R, :].rearrange("p (j f) -> p j f", j=G))

            # schedule: explicit tile map, ascending groups per tile.
            # bands: 24 -> g12, g4; 16 -> g13, g5; 8 -> g14, g6;
            # DMA band (r=1..8) -> g3, g15, g7; q1 groups 8-11 need no band.
            tmap = CFG["tmap"]
            zeroed = set()
            for pre, g in CFG["sched"]:
                if pre is not None:
                    build_band(pre)
                    for b in range(4):
                        if CFG["mz"][b] == "scalar" and b not in zeroed:
                            zeroed.add(b)
                            nc.scalar.activation(
                                out=rings[b][:].rearrange(
                                    "p (j f) -> p j f", j=G),
                                in_=zq[:].unsqueeze(1)
                                .broadcast_to((128, G, F)), func=CPY)
                            break
                do_group(g, rings[tmap[g]])
    nc.compile()
    return nc


def _make_ew():
    ew = np.zeros((128, 256), np.float16)
    ewn = np.zeros((128, 256), np.float16)
    for p in range(128):
        ew[p, p + 96] = 1.0
        ewn[p, p + 96] = -1.0
    return ew, ewn


def _run(left, right, trace=False):
    """left/right: [N, C, H, W] f32. Returns (full_out, exec_time_ns)."""
    if "nc" not in _cached:
        _cached["nc"] = _build_program()
    nc = _cached["nc"]
    left = np.asarray(left)
    right = np.asarray(right)
    ew, ewn = _make_ew()
    in_maps = []
    for k in range(N_CORES):
        sl = slice(k * C_LOC, (k + 1) * C_LOC)
        lt = left[0, sl].transpose(1, 0, 2).reshape(H, F).astype(np.float16)
        rtm = right[0, sl].transpose(1, 0, 2).reshape(H, F).astype(np.float16)
        lp = np.concatenate([np.zeros((LPAD, F), np.float16), lt], axis=0)
        rp = np.concatenate([rtm, np.zeros((RPAD, F), np.float16)], axis=0)
        in_maps.append({"left": np.ascontiguousarray(lp),
                        "right": np.ascontiguousarray(rp),
                        "ew": ew, "ewn": ewn})
    res = run_bass_kernel_spmd(nc, in_maps, core_ids=list(range(N_CORES)),
                               trace=trace)
    K3 = CFG["k3max"]
    parts = []
    for k in range(N_CORES):
        full16 = (res.results[k]["out"].astype(np.float32)
                  .reshape(D, H, C_LOC, W))
        o32 = res.results[k]["out32"].reshape(K3, 32, C_LOC, W)
        for d in range(1, K3):
            full16[d, 0:d] = o32[d, 0:d]
        parts.append(full16.transpose(0, 2, 1, 3))
    full = np.concatenate(parts, axis=1)
    return np.ascontiguousarray(full[None]), res.exec_time_ns


def kernel(left, right):
    out, _ = _run(left, right, trace=False)
    return out
